# revision 5
# baseline (speedup 1.0000x reference)
"""Trainium2 Bass kernel for nn_ComplexMixture.

Reference:
  output_real[b,n,m] = sum_s w[b,s] * (r[b,s,n]*r[b,s,m] + i[b,s,n]*i[b,s,m])
  output_imag[b,n,m] = sum_s w[b,s] * (i[b,s,n]*r[b,s,m] - r[b,s,n]*i[b,s,m])

Shapes: B=32, S=128, N=256, fp32. w is uniform [0,1) so sqrt(w) is real.

out_r is symmetric and out_i is antisymmetric, so the device only computes
  P = out_r + out_i
and the host recovers out_r = (P + P^T)/2, out_i = (P - P^T)/2.
The host pre-scales the inputs: Yr = sqrt(w)[:,None]*r, Yi = sqrt(w)[:,None]*i.
With U = Yr - Yi, V = Yr + Yi:
  P[n,m] = sum_s Yr[s,n]*U[s,m] + Yi[s,n]*V[s,m]
i.e. per 128-row output chunk c:  P_c = Yr_c.T @ U + Yi_c.T @ V  (PSUM accum).

v4 (from 19.5us v3; baseline 24.5us). All bf16 I/O (rel err 4.3e-3 vs 2e-2
gate), PSUM accumulation fp32. Changes:
  - no PE warmup: on this part the HAM clock gate needs ~9us of sustained
    activity to release (measured v1/v3), which a ~15us kernel never reaches.
    All matmuls run at 1.2 GHz: 213ns per 256-wide mm, 3.4us total.
  - 5 input kicks: b0 alone in the first (small) chunk per HWDGE ring so the
    matmul stream starts earliest; b1b2 second chunk; b3 whole on SWDGE.
  - interleaved PSUM groups per batch: both U-matmuls (c0,c1) issue right
    after the vector `sub`, while the `add` (V) runs in parallel.
  - per-chunk PSUM->SBUF casts (8x [128,256], split vector/scalar) start as
    soon as each accumulation group closes instead of waiting for the batch.
  - last batch drains as two independent half-chains: each cast's engine
    kicks its own 64KB DMA on its own hot queue (no cross-engine join).
"""

import os

import numpy as np

import concourse.bass as bass
import concourse.mybir as mybir
import concourse.tile as tile
from concourse import bacc
from concourse.bass_utils import run_bass_kernel_spmd

B, S, N = 32, 128, 256
NCORES = 8
BPC = B // NCORES  # batches per core
W = 2 * N  # columns per batch block
XCOL = BPC * W  # 2048 bf16 per partition row

F32 = mybir.dt.float32
BF16 = mybir.dt.bfloat16

LAST_RESULTS = None  # stashed BassKernelResults for test harness introspection


def build_nc() -> bass.Bass:
    nc = bacc.Bacc(num_swdge_queues=1)
    xin = nc.dram_tensor("xpack", [S, XCOL], BF16, kind="ExternalInput")
    out = nc.dram_tensor("out_all", [128, XCOL], BF16, kind="ExternalOutput")

    with tile.TileContext(nc) as tc:
        with (
            tc.tile_pool(name="io", bufs=1) as io_pool,
            tc.tile_pool(name="uv", bufs=BPC) as uv_pool,
            tc.tile_pool(name="op", bufs=1) as out_pool,
            tc.tile_pool(name="ps", bufs=BPC, space="PSUM") as ps_pool,
        ):
            X_all = io_pool.tile([S, XCOL], BF16, tag="X", name="X_all")

            # Input: b0 first in its own small chunk on both HWDGE rings
            # (partition-split, 1KB packets), then b1b2 (2KB packets), b3 on
            # SWDGE. Expected ready order: b0, b3, b1, b2 (b1b2 share a sem).
            nc.sync.dma_start(out=X_all[0:64, 0:W], in_=xin[0:64, 0:W])
            nc.scalar.dma_start(out=X_all[64:128, 0:W], in_=xin[64:128, 0:W])
            nc.gpsimd.dma_start(out=X_all[:, 3 * W : 4 * W], in_=xin[:, 3 * W : 4 * W])
            nc.sync.dma_start(out=X_all[0:64, W : 3 * W], in_=xin[0:64, W : 3 * W])
            nc.scalar.dma_start(out=X_all[64:128, W : 3 * W], in_=xin[64:128, W : 3 * W])

            O = out_pool.tile([128, XCOL], BF16, tag="O", name="O_all")

            # Per-batch UV prep on vector; program order groups subs/adds so
            # the scheduler can run them as inputs arrive.
            UVs = {}
            for b in (0, 1, 3, 2):
                X = X_all[:, b * W : (b + 1) * W]
                UV = uv_pool.tile([S, W], BF16, tag="UV", name=f"UV{b}")
                nc.vector.tensor_sub(UV[:, 0:N], X[:, 0:N], X[:, N:W])
                nc.vector.tensor_add(UV[:, N:W], X[:, 0:N], X[:, N:W])
                UVs[b] = UV

            # cast engine per batch-chunk: (engine, out-kick engine or None)
            # b0 -> scalar casts + scalar kick; b1 -> vector casts + sync kick
            # b3 -> scalar casts + scalar kick; b2 (tail) -> split chains.
            for b in (0, 1, 3, 2):
                X = X_all[:, b * W : (b + 1) * W]
                UV = UVs[b]
                ps = ps_pool.tile([128, W], F32, tag="ps", name=f"ps{b}")
                for c in range(2):
                    osl = slice(c * N, (c + 1) * N)
                    nc.tensor.matmul(ps[:, osl], lhsT=X[:, c * 128 : c * 128 + 128],
                                     rhs=UV[:, 0:N], start=True, stop=False)
                    nc.tensor.matmul(ps[:, osl], lhsT=X[:, N + c * 128 : N + c * 128 + 128],
                                     rhs=UV[:, N:W], start=False, stop=True)

                o0 = slice(b * W, b * W + N)
                o1 = slice(b * W + N, (b + 1) * W)
                if b == 2:
                    # Tail: two independent cast->kick chains on two queues.
                    nc.vector.tensor_copy(O[:, o0], ps[:, 0:N])
                    nc.sync.dma_start(out=out[:, o0], in_=O[:, o0])
                    nc.scalar.copy(out=O[:, o1], in_=ps[:, N:W])
                    nc.scalar.dma_start(out=out[:, o1], in_=O[:, o1])
                elif b == 1:
                    nc.vector.tensor_copy(O[:, o0], ps[:, 0:N])
                    nc.vector.tensor_copy(O[:, o1], ps[:, N:W])
                    nc.sync.dma_start(out=out[:, b * W : (b + 1) * W], in_=O[:, b * W : (b + 1) * W])
                else:  # b0, b3 on scalar
                    nc.scalar.copy(out=O[:, o0], in_=ps[:, 0:N])
                    nc.scalar.copy(out=O[:, o1], in_=ps[:, N:W])
                    nc.scalar.dma_start(out=out[:, b * W : (b + 1) * W], in_=O[:, b * W : (b + 1) * W])
    nc.compile()
    return nc


def kernel(**inputs: np.ndarray):
    global LAST_RESULTS
    import ml_dtypes

    r = np.asarray(inputs["input_real"], dtype=np.float32)
    i = np.asarray(inputs["input_imag"], dtype=np.float32)
    w = np.ascontiguousarray(np.asarray(inputs["weight"], dtype=np.float32))
    assert r.shape == (B, S, N) and i.shape == (B, S, N) and w.shape == (B, S)

    # [B, 2, S, N] -> per-core [S, (b t n)] batch-major blocks, bf16
    sws = np.sqrt(w)  # [B, S]
    xin = (np.stack([r, i], axis=1) * sws[:, None, :, None]).astype(ml_dtypes.bfloat16)

    in_maps = []
    for c in range(NCORES):
        sl = slice(c * BPC, (c + 1) * BPC)
        xpack = np.transpose(xin[sl], (2, 0, 1, 3)).reshape(S, XCOL)
        in_maps.append({"xpack": np.ascontiguousarray(xpack)})

    nc = build_nc()
    res = run_bass_kernel_spmd(nc, in_maps, core_ids=list(range(NCORES)))
    LAST_RESULTS = res

    # out_all[core] is [128, (b c m)] bf16; P[b, c*128+p, m] = out[p, b*512 + c*256 + m]
    out_all = np.stack(
        [np.asarray(res.results[c]["out_all"]) for c in range(NCORES)], axis=0
    ).astype(np.float32)  # [NCORES, 128, XCOL]
    out_all = out_all.reshape(NCORES, 128, BPC, 2, N)
    P = np.transpose(out_all, (0, 2, 3, 1, 4)).reshape(B, N, N)
    Pt = np.transpose(P, (0, 2, 1))
    out_r = (P + Pt) * np.float32(0.5)
    out_i = (P - Pt) * np.float32(0.5)
    return (np.ascontiguousarray(out_r), np.ascontiguousarray(out_i))


# revision 6
# speedup vs baseline: 1.0663x; 1.0663x over previous
"""Trainium2 Bass kernel for nn_ComplexMixture.

Reference:
  output_real[b,n,m] = sum_s w[b,s] * (r[b,s,n]*r[b,s,m] + i[b,s,n]*i[b,s,m])
  output_imag[b,n,m] = sum_s w[b,s] * (i[b,s,n]*r[b,s,m] - r[b,s,n]*i[b,s,m])

Shapes: B=32, S=128, N=256, fp32. w is uniform [0,1) so sqrt(w) is real.

out_r is symmetric and out_i is antisymmetric, so the device only computes
  P = out_r + out_i
and the host recovers out_r = (P + P^T)/2, out_i = (P - P^T)/2.
The host pre-scales the inputs: Yr = sqrt(w)[:,None]*r, Yi = sqrt(w)[:,None]*i.
With U = Yr - Yi, V = Yr + Yi:
  P[n,m] = sum_s Yr[s,n]*U[s,m] + Yi[s,n]*V[s,m]
i.e. per 128-row output chunk c:  P_c = Yr_c.T @ U + Yi_c.T @ V  (PSUM accum).

v4 (from 19.5us v3; baseline 24.5us). All bf16 I/O (rel err 4.3e-3 vs 2e-2
gate), PSUM accumulation fp32. Changes:
  - no PE warmup: on this part the HAM clock gate needs ~9us of sustained
    activity to release (measured v1/v3), which a ~15us kernel never reaches.
    All matmuls run at 1.2 GHz: 213ns per 256-wide mm, 3.4us total.
  - 5 input kicks: b0 alone in the first (small) chunk per HWDGE ring so the
    matmul stream starts earliest; b1b2 second chunk; b3 whole on SWDGE.
  - interleaved PSUM groups per batch: both U-matmuls (c0,c1) issue right
    after the vector `sub`, while the `add` (V) runs in parallel.
  - per-chunk PSUM->SBUF casts (8x [128,256], split vector/scalar) start as
    soon as each accumulation group closes instead of waiting for the batch.
  - last batch drains as two independent half-chains: each cast's engine
    kicks its own 64KB DMA on its own hot queue (no cross-engine join).
"""

import os

import numpy as np

import concourse.bass as bass
import concourse.mybir as mybir
import concourse.tile as tile
from concourse import bacc
from concourse.bass_utils import run_bass_kernel_spmd

B, S, N = 32, 128, 256
NCORES = 8
BPC = B // NCORES  # batches per core
W = 2 * N  # columns per batch block
XCOL = BPC * W  # 2048 bf16 per partition row

F32 = mybir.dt.float32
BF16 = mybir.dt.bfloat16

LAST_RESULTS = None  # stashed BassKernelResults for test harness introspection


def build_nc() -> bass.Bass:
    nc = bacc.Bacc(num_swdge_queues=1)
    xin = nc.dram_tensor("xpack", [S, XCOL], BF16, kind="ExternalInput")
    out = nc.dram_tensor("out_all", [128, XCOL], BF16, kind="ExternalOutput")

    with tile.TileContext(nc) as tc:
        with (
            tc.tile_pool(name="io", bufs=1) as io_pool,
            tc.tile_pool(name="uv", bufs=BPC) as uv_pool,
            tc.tile_pool(name="op", bufs=1) as out_pool,
            tc.tile_pool(name="ps", bufs=BPC, space="PSUM") as ps_pool,
        ):
            X_all = io_pool.tile([S, XCOL], BF16, tag="X", name="X_all")

            # Input: pair0 = b0b1 split by partition halves on the two HWDGE
            # rings (2KB packets); b2 and b3 as separate SWDGE kicks so each
            # gets its own completion semaphore. Ready order: b0b1, b2, b3.
            nc.sync.dma_start(out=X_all[0:64, 0 : 2 * W], in_=xin[0:64, 0 : 2 * W])
            nc.scalar.dma_start(out=X_all[64:128, 0 : 2 * W], in_=xin[64:128, 0 : 2 * W])
            nc.gpsimd.dma_start(out=X_all[:, 2 * W : 3 * W], in_=xin[:, 2 * W : 3 * W])
            nc.gpsimd.dma_start(out=X_all[:, 3 * W : 4 * W], in_=xin[:, 3 * W : 4 * W])

            O = out_pool.tile([128, XCOL], BF16, tag="O", name="O_all")

            # All UV prep on vector, emitted before any cast so the in-order
            # engine streams UVs back-to-back as inputs arrive.
            UVs = {}
            for b in range(BPC):
                X = X_all[:, b * W : (b + 1) * W]
                UV = uv_pool.tile([S, W], BF16, tag="UV", name=f"UV{b}")
                nc.vector.tensor_sub(UV[:, 0:N], X[:, 0:N], X[:, N:W])
                nc.vector.tensor_add(UV[:, N:W], X[:, 0:N], X[:, N:W])
                UVs[b] = UV

            for b in range(BPC):
                X = X_all[:, b * W : (b + 1) * W]
                UV = UVs[b]
                ps = ps_pool.tile([128, W], F32, tag="ps", name=f"ps{b}")
                for c in range(2):
                    osl = slice(c * N, (c + 1) * N)
                    nc.tensor.matmul(ps[:, osl], lhsT=X[:, c * 128 : c * 128 + 128],
                                     rhs=UV[:, 0:N], start=True, stop=False)
                    nc.tensor.matmul(ps[:, osl], lhsT=X[:, N + c * 128 : N + c * 128 + 128],
                                     rhs=UV[:, N:W], start=False, stop=True)

                o0 = slice(b * W, b * W + N)
                o1 = slice(b * W + N, (b + 1) * W)
                oall = slice(b * W, (b + 1) * W)
                if b == 0:
                    # scalar casts early (free after its in-kick), scalar kick
                    nc.scalar.copy(out=O[:, o0], in_=ps[:, 0:N])
                    nc.scalar.copy(out=O[:, o1], in_=ps[:, N:W])
                    nc.scalar.dma_start(out=out[:, oall], in_=O[:, oall])
                elif b == 1:
                    nc.scalar.copy(out=O[:, o0], in_=ps[:, 0:N])
                    nc.scalar.copy(out=O[:, o1], in_=ps[:, N:W])
                    nc.sync.dma_start(out=out[:, oall], in_=O[:, oall])
                elif b == 2:
                    # vector is done with UVs by now
                    nc.vector.tensor_copy(O[:, o0], ps[:, 0:N])
                    nc.vector.tensor_copy(O[:, o1], ps[:, N:W])
                    nc.gpsimd.dma_start(out=out[:, oall], in_=O[:, oall])
                else:
                    # Tail: two independent cast->kick chains on two queues.
                    nc.scalar.copy(out=O[:, o0], in_=ps[:, 0:N])
                    nc.scalar.dma_start(out=out[:, o0], in_=O[:, o0])
                    nc.vector.tensor_copy(O[:, o1], ps[:, N:W])
                    nc.sync.dma_start(out=out[:, o1], in_=O[:, o1])
    nc.compile()
    return nc


def kernel(**inputs: np.ndarray):
    global LAST_RESULTS
    import ml_dtypes

    r = np.asarray(inputs["input_real"], dtype=np.float32)
    i = np.asarray(inputs["input_imag"], dtype=np.float32)
    w = np.ascontiguousarray(np.asarray(inputs["weight"], dtype=np.float32))
    assert r.shape == (B, S, N) and i.shape == (B, S, N) and w.shape == (B, S)

    # [B, 2, S, N] -> per-core [S, (b t n)] batch-major blocks, bf16
    sws = np.sqrt(w)  # [B, S]
    xin = (np.stack([r, i], axis=1) * sws[:, None, :, None]).astype(ml_dtypes.bfloat16)

    in_maps = []
    for c in range(NCORES):
        sl = slice(c * BPC, (c + 1) * BPC)
        xpack = np.transpose(xin[sl], (2, 0, 1, 3)).reshape(S, XCOL)
        in_maps.append({"xpack": np.ascontiguousarray(xpack)})

    nc = build_nc()
    res = run_bass_kernel_spmd(nc, in_maps, core_ids=list(range(NCORES)))
    LAST_RESULTS = res

    # out_all[core] is [128, (b c m)] bf16; P[b, c*128+p, m] = out[p, b*512 + c*256 + m]
    out_all = np.stack(
        [np.asarray(res.results[c]["out_all"]) for c in range(NCORES)], axis=0
    ).astype(np.float32)  # [NCORES, 128, XCOL]
    out_all = out_all.reshape(NCORES, 128, BPC, 2, N)
    P = np.transpose(out_all, (0, 2, 3, 1, 4)).reshape(B, N, N)
    Pt = np.transpose(P, (0, 2, 1))
    out_r = (P + Pt) * np.float32(0.5)
    out_i = (P - Pt) * np.float32(0.5)
    return (np.ascontiguousarray(out_r), np.ascontiguousarray(out_i))


# revision 8
# speedup vs baseline: 1.0767x; 1.0098x over previous
"""Trainium2 Bass kernel for nn_ComplexMixture.

Reference:
  output_real[b,n,m] = sum_s w[b,s] * (r[b,s,n]*r[b,s,m] + i[b,s,n]*i[b,s,m])
  output_imag[b,n,m] = sum_s w[b,s] * (i[b,s,n]*r[b,s,m] - r[b,s,n]*i[b,s,m])

Shapes: B=32, S=128, N=256, fp32. w is uniform [0,1) so sqrt(w) is real.

out_r is symmetric and out_i is antisymmetric, so the device only computes
  P = out_r + out_i
and the host recovers out_r = (P + P^T)/2, out_i = (P - P^T)/2.
The host pre-scales the inputs: Yr = sqrt(w)[:,None]*r, Yi = sqrt(w)[:,None]*i.
With U = Yr - Yi, V = Yr + Yi:
  P[n,m] = sum_s Yr[s,n]*U[s,m] + Yi[s,n]*V[s,m]
i.e. per 128-row output chunk c:  P_c = Yr_c.T @ U + Yi_c.T @ V  (PSUM accum).

v4 (from 19.5us v3; baseline 24.5us). All bf16 I/O (rel err 4.3e-3 vs 2e-2
gate), PSUM accumulation fp32. Changes:
  - no PE warmup: on this part the HAM clock gate needs ~9us of sustained
    activity to release (measured v1/v3), which a ~15us kernel never reaches.
    All matmuls run at 1.2 GHz: 213ns per 256-wide mm, 3.4us total.
  - 5 input kicks: b0 alone in the first (small) chunk per HWDGE ring so the
    matmul stream starts earliest; b1b2 second chunk; b3 whole on SWDGE.
  - interleaved PSUM groups per batch: both U-matmuls (c0,c1) issue right
    after the vector `sub`, while the `add` (V) runs in parallel.
  - per-chunk PSUM->SBUF casts (8x [128,256], split vector/scalar) start as
    soon as each accumulation group closes instead of waiting for the batch.
  - last batch drains as two independent half-chains: each cast's engine
    kicks its own 64KB DMA on its own hot queue (no cross-engine join).
"""

import os

import numpy as np

import concourse.bass as bass
import concourse.mybir as mybir
import concourse.tile as tile
from concourse import bacc
from concourse.bass_utils import run_bass_kernel_spmd

B, S, N = 32, 128, 256
NCORES = 8
BPC = B // NCORES  # batches per core
W = 2 * N  # columns per batch block
XCOL = BPC * W  # 2048 bf16 per partition row

F32 = mybir.dt.float32
BF16 = mybir.dt.bfloat16

LAST_RESULTS = None  # stashed BassKernelResults for test harness introspection


def build_nc() -> bass.Bass:
    nc = bacc.Bacc(num_swdge_queues=1)
    xin = nc.dram_tensor("xpack", [S, XCOL], BF16, kind="ExternalInput")
    out = nc.dram_tensor("out_all", [128, XCOL], BF16, kind="ExternalOutput")

    with tile.TileContext(nc) as tc:
        with (
            tc.tile_pool(name="io", bufs=1) as io_pool,
            tc.tile_pool(name="uv", bufs=BPC) as uv_pool,
            tc.tile_pool(name="op", bufs=1) as out_pool,
            tc.tile_pool(name="ps", bufs=BPC, space="PSUM") as ps_pool,
        ):
            X_all = io_pool.tile([S, XCOL], BF16, tag="X", name="X_all")

            # Input: pair0 = b0b1 split by partition halves on the two HWDGE
            # rings, pair1 = b2b3 whole on SWDGE (all 2KB packets).
            nc.sync.dma_start(out=X_all[0:64, 0 : 2 * W], in_=xin[0:64, 0 : 2 * W])
            nc.scalar.dma_start(out=X_all[64:128, 0 : 2 * W], in_=xin[64:128, 0 : 2 * W])
            nc.gpsimd.dma_start(out=X_all[:, 2 * W : 4 * W], in_=xin[:, 2 * W : 4 * W])

            O = out_pool.tile([128, XCOL], BF16, tag="O", name="O_all")

            # All UV prep on vector, emitted before any cast so the in-order
            # engine streams UVs back-to-back as inputs arrive.
            UVs = {}
            for b in range(BPC):
                X = X_all[:, b * W : (b + 1) * W]
                UV = uv_pool.tile([S, W], BF16, tag="UV", name=f"UV{b}")
                nc.vector.tensor_sub(UV[:, 0:N], X[:, 0:N], X[:, N:W])
                nc.vector.tensor_add(UV[:, N:W], X[:, 0:N], X[:, N:W])
                UVs[b] = UV

            for b in range(BPC):
                X = X_all[:, b * W : (b + 1) * W]
                UV = UVs[b]
                ps = ps_pool.tile([128, W], F32, tag="ps", name=f"ps{b}")
                for c in range(2):
                    osl = slice(c * N, (c + 1) * N)
                    nc.tensor.matmul(ps[:, osl], lhsT=X[:, c * 128 : c * 128 + 128],
                                     rhs=UV[:, 0:N], start=True, stop=False)
                    nc.tensor.matmul(ps[:, osl], lhsT=X[:, N + c * 128 : N + c * 128 + 128],
                                     rhs=UV[:, N:W], start=False, stop=True)

                o0 = slice(b * W, b * W + N)
                o1 = slice(b * W + N, (b + 1) * W)
                oall = slice(b * W, (b + 1) * W)
                if b == 0:
                    # scalar casts (free after its in-kick) + scalar kick
                    nc.scalar.copy(out=O[:, o0], in_=ps[:, 0:N])
                    nc.scalar.copy(out=O[:, o1], in_=ps[:, N:W])
                    nc.scalar.dma_start(out=out[:, oall], in_=O[:, oall])
                elif b == 1:
                    # vector casts (after UVs) + sync kick
                    nc.vector.tensor_copy(O[:, o0], ps[:, 0:N])
                    nc.vector.tensor_copy(O[:, o1], ps[:, N:W])
                    nc.sync.dma_start(out=out[:, oall], in_=O[:, oall])
                elif b == 2:
                    nc.scalar.copy(out=O[:, o0], in_=ps[:, 0:N])
                    nc.scalar.copy(out=O[:, o1], in_=ps[:, N:W])
                    nc.sync.dma_start(out=out[:, oall], in_=O[:, oall])
                else:
                    # Tail: two independent cast->kick chains on two queues.
                    nc.scalar.copy(out=O[:, o0], in_=ps[:, 0:N])
                    nc.scalar.dma_start(out=out[:, o0], in_=O[:, o0])
                    nc.vector.tensor_copy(O[:, o1], ps[:, N:W])
                    nc.sync.dma_start(out=out[:, o1], in_=O[:, o1])
    nc.compile()
    return nc


def kernel(**inputs: np.ndarray):
    global LAST_RESULTS
    import ml_dtypes

    r = np.asarray(inputs["input_real"], dtype=np.float32)
    i = np.asarray(inputs["input_imag"], dtype=np.float32)
    w = np.ascontiguousarray(np.asarray(inputs["weight"], dtype=np.float32))
    assert r.shape == (B, S, N) and i.shape == (B, S, N) and w.shape == (B, S)

    # [B, 2, S, N] -> per-core [S, (b t n)] batch-major blocks, bf16
    sws = np.sqrt(w)  # [B, S]
    xin = (np.stack([r, i], axis=1) * sws[:, None, :, None]).astype(ml_dtypes.bfloat16)

    in_maps = []
    for c in range(NCORES):
        sl = slice(c * BPC, (c + 1) * BPC)
        xpack = np.transpose(xin[sl], (2, 0, 1, 3)).reshape(S, XCOL)
        in_maps.append({"xpack": np.ascontiguousarray(xpack)})

    nc = build_nc()
    res = run_bass_kernel_spmd(nc, in_maps, core_ids=list(range(NCORES)))
    LAST_RESULTS = res

    # out_all[core] is [128, (b c m)] bf16; P[b, c*128+p, m] = out[p, b*512 + c*256 + m]
    out_all = np.stack(
        [np.asarray(res.results[c]["out_all"]) for c in range(NCORES)], axis=0
    ).astype(np.float32)  # [NCORES, 128, XCOL]
    out_all = out_all.reshape(NCORES, 128, BPC, 2, N)
    P = np.transpose(out_all, (0, 2, 3, 1, 4)).reshape(B, N, N)
    Pt = np.transpose(P, (0, 2, 1))
    out_r = (P + Pt) * np.float32(0.5)
    out_i = (P - Pt) * np.float32(0.5)
    return (np.ascontiguousarray(out_r), np.ascontiguousarray(out_i))
